# revision 6
# baseline (speedup 1.0000x reference)
"""Bass/Trainium2 kernel for a 2-layer LSTM (B=512, T=2048, I=3, H=64).

Returns the final hidden state of layer 2, shape (512, 64) fp32.

v5: raw Bass, two independent 32-batch chains per core (see kernel_v4), with
the cell DVE work collapsed to THREE ops via a layout trick: per chain one
f32 tensor C[128, 160] = [to | ti | tf | tg | c2x] (32 cols each), gates
written by ONE tanh ACT from PSUM blocks ordered [o|i|f|g].  Then:

  uw = ([ti|tf] + 1) * [tg|c2x]   -> [u|w] = [2ig | 4fc]   (one 64-wide STT)
  c2x' = 0.5*w + u                -> written back into C's c2x column
  tc   = tanh(0.5*c2x')           (ACT)
  st   = (to + 1) * tc            -> [2h1; 2h2]            (STT, f16 out)

Chain per round: st-sem -> 4 h-MMs -> gates-ACT -> uw -> c -> tc-ACT -> st.
c2x stays f32 (f16 cell state loses ~0.5 rel err over T=2048).
"""

import numpy as np

B, T, I, H = 512, 2048, 3, 64
NCORES = 8
BL = B // NCORES  # 64 batch per core
HB = BL // 2  # 32 batch per chain
CH = 64  # timesteps per x-chunk DMA
NXR = 3  # x-chunk ring buffers

_CACHE = {}

# PSUM/C gate block order is [o, i, f, g]; PyTorch weight rows are [i, f, g, o]
_BLK = ((3, 0.5), (0, 0.5), (1, 0.5), (2, 1.0))  # (row-block, tanh-trick scale)


def _prep_weights(W_ih0, W_hh0, b_ih0, b_hh0, W_ih1, W_hh1, b_ih1, b_hh1):
    b0 = b_ih0 + b_hh0
    b1 = b_ih1 + b_hh1
    w4x = np.zeros((4, 512), np.float32)
    wh = np.zeros((128, 512), np.float32)
    for G, (rb, s) in enumerate(_BLK):
        gs = slice(rb * 64, (rb + 1) * 64)
        c0 = G * 128
        c1 = G * 128 + 64
        w4x[0:3, c0 : c0 + 64] = s * W_ih0[gs, :].T
        w4x[3, c0 : c0 + 64] = s * b0[gs]
        w4x[3, c1 : c1 + 64] = s * b1[gs]
        wh[0:64, c0 : c0 + 64] = 0.5 * s * W_hh0[gs, :].T
        wh[0:64, c1 : c1 + 64] = 0.5 * s * W_ih1[gs, :].T
        wh[64:128, c1 : c1 + 64] = 0.5 * s * W_hh1[gs, :].T
    w4x0 = w4x.copy()
    w4x0[3, :] = 0.0
    for G, (rb, s) in enumerate(_BLK):
        w4x0[3, G * 128 : G * 128 + 64] = s * b0[rb * 64 : (rb + 1) * 64]
    return (
        w4x.astype(np.float16),
        w4x0.astype(np.float16),
        wh.astype(np.float16),
    )


def build_program(t_steps=T):
    import concourse.bass as bass
    from concourse import bacc, mybir

    f32 = mybir.dt.float32
    f16 = mybir.dt.float16
    Tanh = mybir.ActivationFunctionType.Tanh
    ADD = mybir.AluOpType.add
    MULT = mybir.AluOpType.mult

    nc = bacc.Bacc("TRN2", target_bir_lowering=False, debug=False)

    xt_d = nc.dram_tensor("xt", [4, t_steps * BL], f16, kind="ExternalInput")
    w4x_d = nc.dram_tensor("w4x", [4, 512], f16, kind="ExternalInput")
    w4x0_d = nc.dram_tensor("w4x0", [4, 512], f16, kind="ExternalInput")
    wh_d = nc.dram_tensor("wh", [128, 512], f16, kind="ExternalInput")
    out_d = nc.dram_tensor("out", [64, BL], f32, kind="ExternalOutput")

    n_chunks = (t_steps + CH - 1) // CH
    n_rounds = t_steps + 1

    w4x = nc.alloc_sbuf_tensor("w4xs", [4, 512], f16)
    w4x0 = nc.alloc_sbuf_tensor("w4x0s", [4, 512], f16)
    wh = nc.alloc_sbuf_tensor("whs", [128, 512], f16)
    st = nc.alloc_sbuf_tensor("sts", [128, BL], f16)
    xb = nc.alloc_sbuf_tensor("xbs", [4, NXR * CH * BL], f16)
    ob = nc.alloc_sbuf_tensor("obs", [128, BL], f32)
    C = [nc.alloc_sbuf_tensor(f"cc{x}", [128, 5 * HB], f32) for x in "ab"]
    UW = [nc.alloc_sbuf_tensor(f"uw{x}", [128, 2 * HB], f32) for x in "ab"]
    TC = [nc.alloc_sbuf_tensor(f"tc{x}", [128, HB], f16) for x in "ab"]
    ps = [nc.alloc_psum_tensor(f"ps{x}", [128, 3 * 512], f32) for x in "ab"]

    sem_w = nc.alloc_semaphore("sem_w")
    sem_x = nc.alloc_semaphore("sem_x")
    sem_out = nc.alloc_semaphore("sem_out")
    sem_done = nc.alloc_semaphore("sem_done")
    sem_mm = [nc.alloc_semaphore(f"sem_mm{x}") for x in "ab"]
    sem_mmo = [nc.alloc_semaphore(f"sem_mmo{x}") for x in "ab"]
    sem_o = [nc.alloc_semaphore(f"sem_o{x}") for x in "ab"]
    sem_act = [nc.alloc_semaphore(f"sem_act{x}") for x in "ab"]
    sem_uw = [nc.alloc_semaphore(f"sem_uw{x}") for x in "ab"]
    sem_c = [nc.alloc_semaphore(f"sem_c{x}") for x in "ab"]
    sem_tc = [nc.alloc_semaphore(f"sem_tc{x}") for x in "ab"]
    sem_st = [nc.alloc_semaphore(f"sem_st{x}") for x in "ab"]

    stv = [st[0:128, 0:HB], st[0:128, HB:BL]]

    def xsl(r, h):
        t = min(r, t_steps - 1)
        ci, off = divmod(t, CH)
        base = (ci % NXR) * CH * BL + off * BL + h * HB
        return xb[0:4, base : base + HB]

    def psv(h, r, lo, hi):
        base = (r % 3) * 512
        return ps[h][0:128, base + lo : base + hi]

    with nc.Block() as blk:

        @blk.sync
        def _(sync: bass.BassEngine):
            sync.dma_start(w4x[:, :], w4x_d.ap()[:, :]).then_inc(sem_w, 16)
            sync.dma_start(w4x0[:, :], w4x0_d.ap()[:, :]).then_inc(sem_w, 16)
            sync.dma_start(wh[:, :], wh_d.ap()[:, :]).then_inc(sem_w, 16)
            for ci in range(n_chunks):
                lo = ci * CH * BL
                hi = min((ci + 1) * CH, t_steps) * BL
                base = (ci % NXR) * CH * BL
                d = sync.dma_start(
                    xb[0:4, base : base + hi - lo], xt_d.ap()[:, lo:hi]
                ).then_inc(sem_x, 16)
                if ci >= NXR:
                    d._wait_ge(sem_st[1], (ci - NXR + 1) * CH + 1)
            sync.wait_ge(sem_out, 1)
            sync.dma_start(out_d.ap()[:, :], ob[64:128, 0:BL]).then_inc(sem_done, 16)
            sync.wait_ge(sem_done, 16)

        @blk.tensor
        def _(pe: bass.BassEngine):
            pe.wait_ge(sem_w, 48)
            for r in range(n_rounds):
                wx = w4x0 if r == 0 else w4x
                if r % CH == 0:
                    ci = min(r, t_steps - 1) // CH
                    pe.wait_ge(sem_x, (ci + 1) * 16)
                for h in range(2):
                    xr = xsl(r, h)
                    for G in range(4):
                        mm = pe.matmul(
                            psv(h, r, G * HB, (G + 1) * HB),
                            wx[0:4, G * 128 : (G + 1) * 128],
                            xr,
                            start=(G == 0),
                            stop=False,
                        )
                        if G == 0 and r >= 3:
                            mm._wait_ge(sem_act[h], r - 2)
                    for G in range(4):
                        mm = pe.matmul(
                            psv(h, r, G * HB, (G + 1) * HB),
                            wh[0:128, G * 128 : (G + 1) * 128],
                            stv[h],
                            start=False,
                            stop=(G == 3),
                        )
                        if G == 0:
                            mm._wait_ge(sem_st[h], r + 1)
                        if G == 3:
                            mm.then_inc(sem_mm[h], 1)

        @blk.scalar
        def _(act: bass.BassEngine):
            for r in range(n_rounds):
                for h in range(2):
                    act.activation(
                        C[h][0:128, 0 : 4 * HB], psv(h, r, 0, 4 * HB), Tanh
                    )._wait_ge(sem_mm[h], r + 1).then_inc(sem_act[h], 1)
                for h in range(2):
                    act.activation(
                        TC[h][0:128, 0:HB], C[h][0:128, 4 * HB : 5 * HB],
                        Tanh, scale=0.5,
                    )._wait_ge(sem_c[h], r + 2).then_inc(sem_tc[h], 1)

        @blk.vector
        def _(v: bass.BassEngine):
            for h in range(2):
                v.memset(C[h][0:128, 4 * HB : 5 * HB], 0.0).then_inc(sem_c[h], 1)
            v.memset(stv[0], 0.0).then_inc(sem_st[0], 1)
            v.memset(stv[1], 0.0).then_inc(sem_st[1], 1)
            v.wait_ge(sem_c[0], 1)
            v.wait_ge(sem_c[1], 1)
            for r in range(n_rounds):
                for h in range(2):
                    v.scalar_tensor_tensor(
                        UW[h][0:128, 0 : 2 * HB],
                        C[h][0:128, HB : 3 * HB],
                        1.0,
                        C[h][0:128, 3 * HB : 5 * HB],
                        ADD,
                        MULT,
                    )._wait_ge(sem_act[h], r + 1).then_inc(sem_uw[h], 1)
                    v.scalar_tensor_tensor(
                        C[h][0:128, 4 * HB : 5 * HB],
                        UW[h][0:128, HB : 2 * HB],
                        0.5,
                        UW[h][0:128, 0:HB],
                        MULT,
                        ADD,
                    )._wait_ge(sem_uw[h], r + 1).then_inc(sem_c[h], 1)
                for h in range(2):
                    v.scalar_tensor_tensor(
                        stv[h], C[h][0:128, 0:HB], 1.0, TC[h][0:128, 0:HB],
                        ADD, MULT,
                    )._wait_ge(sem_tc[h], r + 1).then_inc(sem_st[h], 1)
            v.wait_ge(sem_st[0], n_rounds + 1)
            v.tensor_scalar_mul(ob[64:128, 0:BL], st[64:128, 0:BL], 0.5)._wait_ge(
                sem_st[1], n_rounds + 1
            ).then_inc(sem_out, 1)

    nc.compile()
    return nc


def _get_program(t_steps=T):
    key = ("prog", t_steps)
    if key not in _CACHE:
        _CACHE[key] = build_program(t_steps)
    return _CACHE[key]


def make_in_maps(inputs, tt=T):
    x = np.asarray(inputs["x"], np.float32)
    w4x, w4x0, wh = _prep_weights(
        *(np.asarray(inputs[k], np.float32) for k in
          ("W_ih0", "W_hh0", "b_ih0", "b_hh0", "W_ih1", "W_hh1", "b_ih1", "b_hh1"))
    )
    in_maps = []
    for c in range(NCORES):
        xc = x[c * BL : (c + 1) * BL, :tt]
        xt = np.ones((4, tt * BL), np.float16)
        xt[0:3] = xc.transpose(2, 1, 0).reshape(3, tt * BL).astype(np.float16)
        in_maps.append({"xt": xt, "w4x": w4x, "w4x0": w4x0, "wh": wh})
    return in_maps


def kernel(x, W_ih0, W_hh0, b_ih0, b_hh0, W_ih1, W_hh1, b_ih1, b_hh1):
    from concourse import bass_utils

    nc = _get_program(T)
    in_maps = make_in_maps(dict(
        x=x, W_ih0=W_ih0, W_hh0=W_hh0, b_ih0=b_ih0, b_hh0=b_hh0,
        W_ih1=W_ih1, W_hh1=W_hh1, b_ih1=b_ih1, b_hh1=b_hh1))

    res = bass_utils.run_bass_kernel_spmd(nc, in_maps, core_ids=list(range(NCORES)))
    outs = [res.results[c]["out"].T for c in range(NCORES)]
    return np.concatenate(outs, axis=0).astype(np.float32)


if __name__ == "__main__":
    rng = np.random.default_rng(0)
    s = 1.0 / np.sqrt(H)
    inputs = {
        "x": rng.standard_normal((B, T, I), np.float32),
        "W_ih0": rng.uniform(-s, s, (4 * H, I)).astype(np.float32),
        "W_hh0": rng.uniform(-s, s, (4 * H, H)).astype(np.float32),
        "b_ih0": rng.uniform(-s, s, 4 * H).astype(np.float32),
        "b_hh0": rng.uniform(-s, s, 4 * H).astype(np.float32),
        "W_ih1": rng.uniform(-s, s, (4 * H, H)).astype(np.float32),
        "W_hh1": rng.uniform(-s, s, (4 * H, H)).astype(np.float32),
        "b_ih1": rng.uniform(-s, s, 4 * H).astype(np.float32),
        "b_hh1": rng.uniform(-s, s, 4 * H).astype(np.float32),
    }
    out = kernel(**inputs)
    print(out.shape, out.dtype, np.abs(out).max())
